# revision 49
# baseline (speedup 1.0000x reference)
"""Sparse-attention TRN2 kernel (bf16/fp32r rewrite).

Reference computation (per batch b):
  pf = normalize(x @ W_pf.T); ns = normalize(x @ W_ns.T); v = x @ W_v.T
  G = pf @ pf.T                                (T x T cosine sims)
  M[u, y] = max_{j<5} G[u, start(y)+j]         (sliding window max, clamped)
  S_pf[x, y] = sum_i w_pf[i] * M[start(x)+i, y]  == (W_band @ M)[x, y]
  S_ns[x, y] = sum_t Q[x, t] * (ns_n[t] . ns_n[y])   with
      Q[x, t] = sum_n w_ns[n] * [inxs[x, n] == t]    (host-precomputed)
  L = S_pf + S_ns + mask(radj);  attn = softmax(L, axis=-1);  out = attn @ v

Kernel computes L.T (y on partitions, x free) so softmax normalization and
the attn@v contraction need no transposes of the T x T tensors.

Speed notes vs the fp32 version:
  - all matmuls run in bf16 or fp32r (1 PE cycle/row at N>=256 vs 4 for fp32)
  - the ns top-k gather (DRAM spill + gpsimd dma_gather) is replaced by a
    host-built Q^T matrix and one accumulating matmul pair
  - sliding max ladder split across DVE (PSUM-reading ops) and gpsimd
    (SBUF-only ops); gpsimd cannot touch PSUM
  - rsqrt for the normalizations is exp(-0.5*ln(x)) so every Activation op
    (Square/Ln/Exp/Copy) lives in one activation table (no table reloads)
"""

import sys

sys.path.insert(0, "/opt/trn_rl_repo")

from contextlib import ExitStack

import numpy as np

import concourse.bacc as bacc
import concourse.bass as bass
import concourse.tile as tile
from concourse import mybir
from concourse._compat import with_exitstack

B, T, C = 32, 256, 128
TNEI = 2
TOPK = 4
NEIGH = 2 * TNEI + 1
N_CORES = 8
BPC = B // N_CORES  # batches per core

F32 = mybir.dt.float32
F32R = mybir.dt.float32r
BF16 = mybir.dt.bfloat16

Act = mybir.ActivationFunctionType
Alu = mybir.AluOpType

NP_BF16 = mybir.dt.np(BF16)


def _blk128(a2d):
    """(T, T)->(128, 2T): out[p, u*T+x] = a2d[x, u*128+p]."""
    return np.ascontiguousarray(
        a2d.T.reshape(2, 128, T).transpose(1, 0, 2).reshape(128, 2 * T)
    )


def host_weights(W_pf, W_ns, W_v, v_pf, g_pf, v_ns, g_ns):
    """Constant (replicated) tensors, all pure layout/small-vector prep."""
    w_pf = (g_pf[0] * v_pf / np.linalg.norm(v_pf)).astype(np.float32)
    w_ns = (g_ns[0] * v_ns / np.linalg.norm(v_ns)).astype(np.float32)

    # Banded weight matrix: W_band[x, u] = w_pf[u - start(x)] on the band.
    start = np.clip(np.arange(T) - TNEI, 0, T - NEIGH)
    W_band = np.zeros((T, T), np.float32)
    for i in range(NEIGH):
        W_band[np.arange(T), start + i] = w_pf[i]
    # WbT[p, u_blk*T + x] = W_band[x, u_blk*128 + p]
    WbT = _blk128(W_band)

    Wcat = np.concatenate([W_pf.T, W_ns.T, W_v.T], axis=1)
    return dict(
        Wcat=np.ascontiguousarray(Wcat).astype(NP_BF16),
        WbT=WbT.astype(NP_BF16),
        identB=np.eye(C, dtype=np.float32).astype(NP_BF16),
        w_ns=w_ns,  # consumed by host_shard (not shipped to the device)
    )


def host_shard(x, radj, inxs, w_ns, core):
    """Per-core input shard: batches [core*BPC, (core+1)*BPC).

    One blob per batch: [xT (256) | radjT (512) | QT (512)] along cols.
    """
    sl = slice(core * BPC, (core + 1) * BPC)
    xT = np.ascontiguousarray(x[sl].transpose(0, 2, 1)).astype(NP_BF16)
    rj = (radj[sl] != 0).astype(np.float32)
    radjT = np.stack([_blk128(rj[i]) for i in range(BPC)]).astype(NP_BF16)
    ix = np.asarray(inxs[sl])
    rows = np.repeat(np.arange(T), TOPK)
    vals = np.tile(w_ns, T)
    QT = np.empty((BPC, 128, 2 * T), np.float32)
    for i in range(BPC):
        Q = np.zeros((T, T), np.float32)
        np.add.at(Q, (rows, ix[i].ravel()), vals)
        QT[i] = _blk128(Q)
    rqb = np.concatenate([radjT, QT.astype(NP_BF16)], axis=2)
    xTp = xT.reshape(BPC // 2, 2, C, T).transpose(0, 2, 1, 3).reshape(
        BPC // 2, C, 2 * T
    )
    rqp = rqb.reshape(BPC // 2, 2, 128, 4 * T).transpose(0, 2, 1, 3).reshape(
        BPC // 2, 128, 8 * T
    )
    return dict(xTp=np.ascontiguousarray(xTp), rqp=np.ascontiguousarray(rqp))


@with_exitstack
def emit_kernel(ctx: ExitStack, tc: tile.TileContext, io: dict, bpc: int = BPC):
    nc = tc.nc
    W = 385  # per-token-block width of pjs: [pf(128) | ns(128) | v(128) | 1]

    consts = ctx.enter_context(tc.tile_pool(name="consts", bufs=1))
    inp = ctx.enter_context(tc.tile_pool(name="inp", bufs=4))
    work = ctx.enter_context(tc.tile_pool(name="work", bufs=4))
    pwork = ctx.enter_context(tc.tile_pool(name="pwork", bufs=2))
    small = ctx.enter_context(tc.tile_pool(name="small", bufs=4))
    outp = ctx.enter_context(tc.tile_pool(name="outp", bufs=4))
    ps_pj = ctx.enter_context(tc.tile_pool(name="ps_pj", bufs=2, space="PSUM"))
    ps_g = ctx.enter_context(tc.tile_pool(name="ps_g", bufs=1, space="PSUM"))
    ps_lt = ctx.enter_context(tc.tile_pool(name="ps_lt", bufs=2, space="PSUM"))
    ps_qn = ctx.enter_context(tc.tile_pool(name="ps_qn", bufs=1, space="PSUM"))

    # ---- constants (loaded once) ----
    Wcat = consts.tile([C, 3 * C], BF16)
    WbT = consts.tile([128, 2 * T], BF16)
    identB = consts.tile([C, C], BF16)
    nc.sync.dma_start(Wcat[:], io["Wcat"][:])

    B_ = [dict() for _ in range(bpc)]  # per-batch tile registry
    P_ = [dict() for _ in range(bpc // 2)]  # per-pair tile registry

    def st_loadx(pi, p):
        p["xTp"] = xTp = inp.tile([C, 2 * T], BF16, tag="xTp", name=f"xTp{pi}")
        nc.sync.dma_start(xTp[:], io["xTp"][pi][:])

    def st_loadrq(pi, p):
        p["rqp"] = rqp = inp.tile([128, 8 * T], BF16, tag="rqp", name=f"rqp{pi}")
        eng = nc.scalar if pi % 2 == 0 else nc.sync
        eng.dma_start(rqp[:], io["rqp"][pi][:])

    def xt_blk(i, p, t):
        return p["xTp"][:, (i % 2) * T + t * C : (i % 2) * T + (t + 1) * C]

    def radjT_ap(i, p):
        return p["rqp"][:, (i % 2) * 4 * T : (i % 2) * 4 * T + 2 * T]

    def qt_blk(i, p, t):
        o = (i % 2) * 4 * T + 2 * T
        return p["rqp"][:, o + t * T : o + (t + 1) * T]

    # ---- front: proj, raw bf16 evac (with v|1 cols), squares ----
    def st_front(i, b, p):
        k = i % 2
        b["pj0"] = ps_pj.tile([128, 3 * C], F32, tag="pj0", name=f"pj0_{i}")
        b["pj1"] = ps_pj.tile([128, 3 * C], F32, tag="pj1", name=f"pj1_{i}")
        pj = (b["pj0"], b["pj1"])
        nc.tensor.matmul(pj[0][:], xt_blk(i, p, 0), Wcat[:], start=True, stop=True)
        nc.tensor.matmul(pj[1][:], xt_blk(i, p, 1), Wcat[:], start=True, stop=True)
        # raw bf16 copy of both projection blocks; col W-1 of each block is 1.0
        b["pjs"] = pjs = work.tile([128, 2 * W], BF16, tag="pjs", name=f"pjs{i}")
        nc.vector.tensor_copy(pjs[:, 0 : 3 * C], pj[0][:])
        nc.scalar.copy(pjs[:, W : W + 3 * C], pj[1][:])
        nc.gpsimd.memset(
            bass.AP(pjs.tensor, pjs.offset + 3 * C, [pjs.ap[0], [W, 2], [1, 1]]),
            1.0,
        )
        # squares (scratch out, no accumulator) into the pair tile
        if k == 0:
            p["sqp"] = pwork.tile([128, 4 * T], BF16, tag="sqp", name=f"sqp{i//2}")
        sqp = p["sqp"]
        nc.scalar.activation(
            sqp[:, k * 4 * C : k * 4 * C + 2 * C], pjs[:, 0 : 2 * C], Act.Square
        )
        nc.gpsimd.tensor_tensor(
            sqp[:, k * 4 * C + 2 * C : (k + 1) * 4 * C],
            pjs[:, W : W + 2 * C],
            pjs[:, W : W + 2 * C],
            Alu.mult,
        )

    # ---- pair front tail: one reduce + one rsqrt chain, then normalize ----
    def st_ftail(pi, p):
        sqp = p["sqp"]
        nrm2 = small.tile([128, 8], F32, tag="nrm2", name=f"nrm2_{pi}")
        nc.vector.tensor_reduce(
            nrm2[:],
            bass.AP(sqp.tensor, sqp.offset, [sqp.ap[0], [C, 8], [1, C]]),
            mybir.AxisListType.X,
            Alu.add,
        )
        # rinv = rsqrt(nrm2): quake bit hack + 1 Newton round (rel err <2e-3)
        bits = small.tile([128, 8], mybir.dt.int32, tag="bits", name=f"bits{pi}")
        rt = small.tile([128, 8], F32, tag="rt", name=f"rt{pi}")
        rinv = small.tile([128, 8], F32, tag="rinv", name=f"rinv{pi}")
        nc.vector.tensor_scalar(
            bits[:], nrm2[:].bitcast(mybir.dt.int32), 1, None,
            Alu.logical_shift_right,
        )
        nc.vector.tensor_scalar(
            bits[:], bits[:], -1, 0x5F3759DF, Alu.mult, Alu.add
        )
        y = bits[:].bitcast(F32)
        nc.vector.tensor_tensor(rt[:], y, y, Alu.mult)
        nc.vector.tensor_tensor(rt[:], rt[:], nrm2[:], Alu.mult)
        nc.vector.tensor_scalar(rt[:], rt[:], -0.5, 1.5, Alu.mult, Alu.add)
        nc.vector.tensor_tensor(rinv[:], y, rt[:], Alu.mult)
        # normalize both projections (pjs is SBUF, so these are cheap)
        for k in range(2):
            b = B_[2 * pi + k]
            pjs = b["pjs"]
            i = 2 * pi + k
            b["pfn"] = pfn = work.tile([128, T], BF16, tag="pfn", name=f"pfn{i}")
            b["nsn"] = nsn = work.tile([128, T], BF16, tag="nsn", name=f"nsn{i}")
            for t in range(2):
                nc.vector.tensor_scalar(
                    pfn[:, t * C : (t + 1) * C],
                    pjs[:, t * W : t * W + C],
                    rinv[:, 4 * k + 2 * t : 4 * k + 2 * t + 1],
                    None,
                    Alu.mult,
                )
                nc.vector.tensor_scalar(
                    nsn[:, t * C : (t + 1) * C],
                    pjs[:, t * W + C : t * W + 2 * C],
                    rinv[:, 4 * k + 2 * t + 1 : 4 * k + 2 * t + 2],
                    None,
                    Alu.mult,
                )

    # ---- back half A (per batch): transposes, gram, q ----
    def st_backA(i, b, p):
        k = i % 2
        pfn, nsn = b["pfn"], b["nsn"]
        tp = ps_qn.tile([C, 4 * C], BF16, tag="qn", name=f"tp{i}")
        for t in range(2):
            nc.tensor.transpose(
                tp[:, t * C : (t + 1) * C], pfn[:, t * C : (t + 1) * C], identB[:]
            )
            nc.tensor.transpose(
                tp[:, (2 + t) * C : (3 + t) * C],
                nsn[:, t * C : (t + 1) * C],
                identB[:],
            )
        b["pnT"] = pnT = work.tile([C, 4 * C], BF16, tag="pnT", name=f"pnT{i}")
        nc.scalar.copy(pnT[:], tp[:])

        # pf gram: G[pp, u*T + y] = pf[u*128+pp] . pf[y]
        G = ps_g.tile([128, 2 * T], F32, tag="G", name=f"G{i}")
        for u in range(2):
            nc.tensor.matmul(
                G[:, u * T : (u + 1) * T],
                pnT[:, u * C : (u + 1) * C],
                pnT[:, 0:T],
                start=True,
                stop=True,
            )
        # evac into the pair Gsb tile (frees the G PSUM bank for the twin)
        if k == 0:
            p["Gsb"] = pwork.tile([128, 4 * T], BF16, tag="Gsb", name=f"Gsb{i//2}")
        nc.scalar.copy(p["Gsb"][:, k * 2 * T : (k + 1) * 2 * T], G[:])

        # q[c, x] = sum_t ns_n[t, c] * Q[x, t]
        q = ps_qn.tile([C, T], F32, tag="qn", name=f"q{i}")
        for t in range(2):
            nc.tensor.matmul(
                q[:],
                nsn[:, t * C : (t + 1) * C],
                qt_blk(i, p, t),
                start=(t == 0),
                stop=(t == 1),
            )
        b["qsb"] = qsb = work.tile([C, T], BF16, tag="qsb", name=f"qsb{i}")
        nc.scalar.copy(qsb[:], q[:])

    # ---- pair ladder: sliding-window max over all 4 G blocks at once ----
    def st_ladder(pi, p):
        Gsb = p["Gsb"]
        m1 = pwork.tile([128, 4 * T], BF16, tag="m1", name=f"m1_{pi}")
        m2 = pwork.tile([128, 4 * T], BF16, tag="m2", name=f"m2_{pi}")
        M = pwork.tile([128, 4 * T], BF16, tag="M", name=f"M{pi}")
        p["M"] = M
        nc.vector.tensor_tensor(
            bass.AP(m1.tensor, m1.offset, [m1.ap[0], [T, 4], [1, T - 1]]),
            bass.AP(Gsb.tensor, Gsb.offset, [Gsb.ap[0], [T, 4], [1, T - 1]]),
            bass.AP(Gsb.tensor, Gsb.offset + 1, [Gsb.ap[0], [T, 4], [1, T - 1]]),
            Alu.max,
        )
        nc.vector.tensor_tensor(
            bass.AP(m2.tensor, m2.offset, [m2.ap[0], [T, 4], [1, T - 3]]),
            bass.AP(m1.tensor, m1.offset, [m1.ap[0], [T, 4], [1, T - 3]]),
            bass.AP(m1.tensor, m1.offset + 2, [m1.ap[0], [T, 4], [1, T - 3]]),
            Alu.max,
        )
        nc.vector.tensor_tensor(
            bass.AP(M.tensor, M.offset + 2, [M.ap[0], [T, 4], [1, T - 4]]),
            bass.AP(m2.tensor, m2.offset, [m2.ap[0], [T, 4], [1, T - 4]]),
            bass.AP(m1.tensor, m1.offset + 3, [m1.ap[0], [T, 4], [1, T - 4]]),
            Alu.max,
        )
        nc.gpsimd.tensor_copy(
            bass.AP(M.tensor, M.offset, [M.ap[0], [T, 4], [T - 2, 2], [1, 2]]),
            bass.AP(M.tensor, M.offset + 2, [M.ap[0], [T, 4], [251, 2], [0, 2]]),
        )

    # ---- back half B (per batch): logits, exp ----
    def st_logits(i, b, p):
        k = i % 2
        M = p["M"]
        mo = k * 2 * T
        LT = ps_lt.tile([128, 2 * T], F32, tag="LT", name=f"LT{i}")
        b["LT"] = LT
        for y in range(2):
            off = y * T
            nc.tensor.matmul(
                LT[:, off : off + T],
                M[:, mo + y * C : mo + (y + 1) * C],
                WbT[:, 0:T],
                start=True,
                stop=False,
            )
            nc.tensor.matmul(
                LT[:, off : off + T],
                M[:, mo + T + y * C : mo + T + (y + 1) * C],
                WbT[:, T : 2 * T],
                start=False,
                stop=False,
            )
            nc.tensor.matmul(
                LT[:, off : off + T],
                b["pnT"][:, (2 + y) * C : (3 + y) * C],
                b["qsb"][:],
                start=False,
                stop=True,
            )
        b["PTe"] = PTe = work.tile([128, 2 * T], BF16, tag="PTe", name=f"PTe{i}")
        nc.scalar.activation(PTe[:], LT[:], Act.Exp)

    # ---- per-batch mask, then output ----
    def st_mask(i, b, p):
        b["PT"] = PT = work.tile([128, 2 * T], BF16, tag="PT", name=f"PT{i}")
        nc.vector.tensor_tensor(PT[:], b["PTe"][:], radjT_ap(i, p), Alu.mult)

    def st_out(i, b, p):
        k = i % 2
        PT, pjs = b["PT"], b["pjs"]
        num = ps_qn.tile([128, 2 * (C + 1)], F32, tag="qn", name=f"num{i}")
        for xt in range(2):
            osl = slice(xt * (C + 1), (xt + 1) * (C + 1))
            for y in range(2):
                nc.tensor.matmul(
                    num[:, osl],
                    PT[:, y * T + xt * C : y * T + (xt + 1) * C],
                    pjs[:, y * W + 2 * C : (y + 1) * W],
                    start=(y == 0),
                    stop=(y == 1),
                )
        dinv = small.tile([128, 2], F32, tag="dinv", name=f"dv{i}")
        nc.vector.reciprocal(
            dinv[:],
            bass.AP(num.tensor, num.offset + C, [num.ap[0], [C + 1, 2], [1, 1]]),
        )
        out_sb = outp.tile([128, T], F32, tag="out_sb", name=f"ou{i}")
        for xt in range(2):
            if k == 0:
                nc.scalar.activation(
                    out_sb[:, xt * C : (xt + 1) * C],
                    num[:, xt * (C + 1) : xt * (C + 1) + C],
                    Act.Copy,
                    scale=dinv[:, xt : xt + 1],
                )
            else:
                nc.vector.tensor_scalar(
                    out_sb[:, xt * C : (xt + 1) * C],
                    num[:, xt * (C + 1) : xt * (C + 1) + C],
                    dinv[:, xt : xt + 1],
                    None,
                    Alu.mult,
                )
        od = io["out"][i]
        nc.sync.dma_start(
            bass.AP(od.tensor, od.offset, [[C, 128], [128 * C, 2], [1, C]]),
            bass.AP(out_sb.tensor, out_sb.offset, [out_sb.ap[0], [C, 2], [1, C]]),
        )

    # prefetch all input DMAs (xT blobs first — they gate the projections),
    # then software-pipeline (4-deep front, pair-shared slide-max ladder):
    st_loadx(0, P_[0])
    nc.sync.dma_start(identB[:], io["identB"][:])
    st_loadx(1, P_[1])
    st_loadrq(0, P_[0])  # scalar queue, ahead of WbT
    nc.scalar.dma_start(WbT[:], io["WbT"][:])
    st_loadrq(1, P_[1])  # sync queue
    st_front(0, B_[0], P_[0])
    st_front(1, B_[1], P_[0])
    st_ftail(0, P_[0])
    st_front(2, B_[2], P_[1])
    st_front(3, B_[3], P_[1])
    st_backA(0, B_[0], P_[0])
    st_ftail(1, P_[1])
    st_backA(1, B_[1], P_[0])
    st_ladder(0, P_[0])
    st_logits(0, B_[0], P_[0])
    st_logits(1, B_[1], P_[0])
    st_mask(0, B_[0], P_[0])
    st_backA(2, B_[2], P_[1])
    st_out(0, B_[0], P_[0])
    st_mask(1, B_[1], P_[0])
    st_backA(3, B_[3], P_[1])
    st_ladder(1, P_[1])
    st_out(1, B_[1], P_[0])
    st_logits(2, B_[2], P_[1])
    st_logits(3, B_[3], P_[1])
    st_mask(2, B_[2], P_[1])
    st_out(2, B_[2], P_[1])
    st_mask(3, B_[3], P_[1])
    st_out(3, B_[3], P_[1])


def build_nc(num_cores: int = 1, bpc: int = BPC):
    nc = bacc.Bacc(None, target_bir_lowering=False, debug=False)
    io = {
        "xTp": nc.dram_tensor("xTp", [bpc // 2, C, 2 * T], BF16, kind="ExternalInput"),
        "rqp": nc.dram_tensor(
            "rqp", [bpc // 2, 128, 8 * T], BF16, kind="ExternalInput"
        ),
        "Wcat": nc.dram_tensor("Wcat", [C, 3 * C], BF16, kind="ExternalInput"),
        "WbT": nc.dram_tensor("WbT", [128, 2 * T], BF16, kind="ExternalInput"),
        "identB": nc.dram_tensor("identB", [C, C], BF16, kind="ExternalInput"),
        "out": nc.dram_tensor("out", [bpc, T, C], F32, kind="ExternalOutput"),
    }
    with tile.TileContext(nc, num_cores=num_cores) as tc:
        emit_kernel(tc, io, bpc=bpc)
    nc.compile()
    return nc


# ---------------------------------------------------------------------------
# Runner: full-input kernel() entry point.
# ---------------------------------------------------------------------------
import os
import time

_NC_CACHE = {}
LAST_RESULT = None


def _get_nc():
    if "nc" not in _NC_CACHE:
        _NC_CACHE["nc"] = build_nc(num_cores=N_CORES, bpc=BPC)
    return _NC_CACHE["nc"]


def _prep_in_maps(x, radj, inxs, W_pf, W_ns, W_v, v_pf, g_pf, v_ns, g_ns):
    x = np.asarray(x, np.float32)
    radj = np.asarray(radj, np.int32)
    inxs = np.asarray(inxs)
    consts = host_weights(
        np.asarray(W_pf, np.float32),
        np.asarray(W_ns, np.float32),
        np.asarray(W_v, np.float32),
        np.asarray(v_pf, np.float32),
        np.asarray(g_pf, np.float32),
        np.asarray(v_ns, np.float32),
        np.asarray(g_ns, np.float32),
    )
    w_ns = consts.pop("w_ns")
    in_maps = []
    for core in range(N_CORES):
        m = dict(consts)
        m.update(host_shard(x, radj, inxs, w_ns, core))
        in_maps.append(m)
    return in_maps


def kernel(x, radj, inxs, W_pf, W_ns, W_v, v_pf, g_pf, v_ns, g_ns):
    global LAST_RESULT
    from concourse.bass_utils import run_bass_kernel_spmd

    in_maps = _prep_in_maps(
        x, radj, inxs, W_pf, W_ns, W_v, v_pf, g_pf, v_ns, g_ns
    )
    nc = _get_nc()
    res = run_bass_kernel_spmd(nc, in_maps, list(range(N_CORES)))
    LAST_RESULT = res
    out = np.concatenate([r["out"] for r in res.results], axis=0)
    return np.ascontiguousarray(out).astype(np.float32)


def bench(inputs: dict, iters: int = 64, warmup: int = 8):
    """Amortized per-iteration wall time of the jitted 8-core executable."""
    import jax
    import jax.numpy as jnp
    from jax.sharding import Mesh, PartitionSpec
    from jax.experimental.shard_map import shard_map

    from concourse import bass2jax, mybir as mb

    nc = _get_nc()
    bass2jax.install_neuronx_cc_hook()
    in_maps = _prep_in_maps(**inputs)

    partition_name = nc.partition_id_tensor.name if nc.partition_id_tensor else None
    in_names, out_names, out_avals, zero_outs = [], [], [], []
    for alloc in nc.m.functions[0].allocations:
        if not isinstance(alloc, mb.MemoryLocationSet):
            continue
        name = alloc.memorylocations[0].name
        if alloc.kind == "ExternalInput":
            if name != partition_name:
                in_names.append(name)
        elif alloc.kind == "ExternalOutput":
            out_names.append(name)
            shape = tuple(alloc.tensor_shape)
            dtype = mb.dt.np(alloc.dtype)
            out_avals.append(jax.core.ShapedArray(shape, dtype))
            zero_outs.append(np.zeros(shape, dtype))
    n_params = len(in_names)
    all_in_names = in_names + out_names
    if partition_name is not None:
        all_in_names = all_in_names + [partition_name]

    def _body(*args):
        operands = list(args)
        if partition_name is not None:
            operands.append(bass2jax.partition_id_tensor())
        outs = bass2jax._bass_exec_p.bind(
            *operands,
            out_avals=tuple(out_avals),
            in_names=tuple(all_in_names),
            out_names=tuple(out_names),
            lowering_input_output_aliases=(),
            sim_require_finite=True,
            sim_require_nnan=True,
            nc=nc,
        )
        return tuple(outs)

    devices = jax.devices()[:N_CORES]
    mesh = Mesh(np.asarray(devices), ("core",))
    fn = jax.jit(
        shard_map(
            _body,
            mesh=mesh,
            in_specs=(PartitionSpec("core"),) * (n_params + len(out_names)),
            out_specs=(PartitionSpec("core"),) * len(out_names),
            check_rep=False,
        ),
        keep_unused=True,
    )
    concat_in = [
        np.concatenate([in_maps[c][nm] for c in range(N_CORES)], axis=0)
        for nm in in_names
    ] + [np.concatenate([z] * N_CORES, axis=0) for z in zero_outs]
    dev_in = [jax.device_put(a) for a in concat_in]

    for _ in range(warmup):
        outs = fn(*dev_in)
    jax.block_until_ready(outs)
    t0 = time.perf_counter()
    for _ in range(iters):
        outs = fn(*dev_in)
    jax.block_until_ready(outs)
    t1 = time.perf_counter()
    per_iter_ns = (t1 - t0) / iters * 1e9
    out_np = np.asarray(outs[out_names.index("out")])
    return per_iter_ns, out_np
